# revision 17
# baseline (speedup 1.0000x reference)
"""Trainium2 Bass kernel for nn_DiffeqSolver: RK4 ODE solver with MLP dynamics.

f(y) = tanh(y@W1 + b1)@W2 + b2;  199 RK4 steps over 4096 trajectories, D=128.

Sharding: data-parallel over B=1024 across 8 cores (B_shard=128, N=512
trajectories/core). Per core the 512 trajectories split into 2 independent
streams of 256 so the serial PE->ACT->PE->DVE chain of one stream overlaps
the other. State kept transposed in SBUF as [D=128 partitions, N free] so
matmul contractions (over D, then H) sit on the partition axis.

Precision strategy (HW-validated): f32r (the PE's 1-cycle/row fp32 mode)
stores only 12 mantissa bits, so it is unusable for the state-carrying first
matmul (h = W1^T y) -> that one runs in true fp32 (4 cycles/row). The second
matmul (z = W2^T a) tolerates a 12-bit `a` (tanh output in (-1,1), scaled by
h/6 on use): it runs f32r with W2 split as r12(W2) + residual, two f32r
matmuls, restoring weight precision to ~2^-24. Numpy-simulated end-to-end
error vs fp64: 1.2e-5 relative (fp32 reference envelope: 5.8e-7).

W2 is prescaled by 0.05/6 and 0.05/3 so PSUM banks accumulate the RK4 z-sums
directly; exact per-step h_i enter via DVE scalar immediates (q = h/0.05,
s = 3h/0.05, ...). b1 via the activation bias operand (per H-chunk); b2 via
GPSIMD per-partition adds (Yb = Y + c*b2 from per-step column tables).
Output needs [N, T, D]: PE transpose (fp32, exact) -> PSUM -> ACT/DVE copy
-> SBUF -> one strided DMA per step (contiguous 512B runs per trajectory).
"""

import numpy as np

import concourse.bass as bass
import concourse.mybir as mybir
from concourse import tile
from concourse.bass_utils import run_bass_kernel_spmd

S, B, D, H, T = 4, 1024, 128, 256, 200
N_CORES = 8
B_SHARD = B // N_CORES          # 128
N = S * B_SHARD                 # 512 trajectories per core
NS = 256                        # stream width (2 streams per core)
N_STREAMS = N // NS
F32 = mybir.dt.float32
F32R = mybir.dt.float32r

# "f32r_res": second matmul in f32r with W2 residual compensation (fast).
# "f32": everything in true fp32 (slow, bit-safest).
W2_MODE = "f32r_res"

_prog_cache = {}


def _r(ap):
    return ap.bitcast(F32R)


def r12(x):
    """Host-side f32r rounding: round-to-nearest, keep 11 explicit mantissa
    bits (measured TRN2 f32r storage behavior)."""
    x = np.ascontiguousarray(x, np.float32)
    b = x.view(np.uint32)
    b = (b + np.uint32(0x800)) & np.uint32(0xFFFFF000)
    return b.view(np.float32)


def _build(h_steps, w2_mode):
    nsteps = len(h_steps)
    t_out = nsteps + 1
    use_res = w2_mode == "f32r_res"
    use_f32r = w2_mode in ("f32r", "f32r_res")
    wdt = F32R if use_f32r else F32

    nc = bass.Bass("TRN2", target_bir_lowering=False, debug=False,
                   num_devices=N_CORES)

    x0 = nc.dram_tensor("x0", [S, B_SHARD, D], F32, kind="ExternalInput").ap()
    w1_d = nc.dram_tensor("w1", [D, H], F32, kind="ExternalInput").ap()
    # prescaled W2 chunk tensors: [chunk, 128, D]; main + (optional) residual
    w16_d = nc.dram_tensor("w2s16", [2, 128, D], wdt, kind="ExternalInput").ap()
    w13_d = nc.dram_tensor("w2s13", [2, 128, D], wdt, kind="ExternalInput").ap()
    if use_res:
        w16r_d = nc.dram_tensor("w2s16r", [2, 128, D], wdt,
                                kind="ExternalInput").ap()
        w13r_d = nc.dram_tensor("w2s13r", [2, 128, D], wdt,
                                kind="ExternalInput").ap()
    b1col_d = nc.dram_tensor("b1col", [128, 2], F32, kind="ExternalInput").ap()
    b2h_d = nc.dram_tensor("b2half", [D, nsteps], F32, kind="ExternalInput").ap()
    b2f_d = nc.dram_tensor("b2full", [D, nsteps], F32, kind="ExternalInput").ap()
    ident_d = nc.dram_tensor("ident", [128, 128], F32, kind="ExternalInput").ap()
    yout = nc.dram_tensor("yout", [S, B_SHARD, t_out, D], F32,
                          kind="ExternalOutput").ap()

    AF = mybir.ActivationFunctionType
    OP = mybir.AluOpType

    with tile.TileContext(nc) as tc:
        with (
            tc.tile_pool(name="const", bufs=1) as cpool,
            tc.tile_pool(name="state", bufs=3) as spool,
            tc.tile_pool(name="work", bufs=6) as wpool,
            tc.tile_pool(name="acts", bufs=6) as apool,
            tc.tile_pool(name="outcp", bufs=4) as opool,
            tc.tile_pool(name="ph0", bufs=1, space="PSUM") as ph0_pool,
            tc.tile_pool(name="ph1", bufs=1, space="PSUM") as ph1_pool,
            tc.tile_pool(name="pz1", bufs=1, space="PSUM") as pz1_pool,
            tc.tile_pool(name="pz23", bufs=1, space="PSUM") as pz23_pool,
        ):
            ph_pools = [ph0_pool, ph1_pool]
            # ---- constants ----
            w1_sb = cpool.tile([D, H], F32, tag="w1")
            nc.sync.dma_start(out=w1_sb[:], in_=w1_d)
            w16 = cpool.tile([128, 2, D], F32, tag="w16")
            nc.sync.dma_start(out=(_r(w16[:]) if use_f32r else w16[:]),
                              in_=w16_d.rearrange("c k d -> k c d"))
            w13 = cpool.tile([128, 2, D], F32, tag="w13")
            nc.sync.dma_start(out=(_r(w13[:]) if use_f32r else w13[:]),
                              in_=w13_d.rearrange("c k d -> k c d"))
            if use_res:
                w16r = cpool.tile([128, 2, D], F32, tag="w16r")
                nc.sync.dma_start(out=_r(w16r[:]),
                                  in_=w16r_d.rearrange("c k d -> k c d"))
                w13r = cpool.tile([128, 2, D], F32, tag="w13r")
                nc.sync.dma_start(out=_r(w13r[:]),
                                  in_=w13r_d.rearrange("c k d -> k c d"))
            else:
                w16r = w13r = None
            b1col = cpool.tile([128, 2], F32, tag="b1col")
            nc.sync.dma_start(out=b1col[:], in_=b1col_d)
            b2h = cpool.tile([D, nsteps], F32, tag="b2h")
            nc.sync.dma_start(out=b2h[:], in_=b2h_d)
            b2f = cpool.tile([D, nsteps], F32, tag="b2f")
            nc.sync.dma_start(out=b2f[:], in_=b2f_d)
            ident = cpool.tile([128, 128], F32, tag="ident")
            nc.sync.dma_start(out=ident[:], in_=ident_d)

            # ---- initial state: load [b,d] tiles, t=0 output, transpose ----
            x0v = x0.rearrange("s b d -> (s b) d")  # n = s*128 + b
            cur = []
            for st in range(N_STREAMS):
                y0 = spool.tile([D, NS], F32, tag=f"Y{st}")
                for c in range(NS // 128):
                    n0 = st * NS + c * 128
                    xin = wpool.tile([128, D], F32, tag="xin")
                    nc.sync.dma_start(out=xin[:], in_=x0v[n0:n0 + 128, :])
                    nc.sync.dma_start(
                        out=yout.rearrange("s b t d -> (s b) t d")[
                            n0:n0 + 128, 0, :],
                        in_=xin[:])
                    tp = pz1_pool.tile([128, NS], F32, tag=f"z1_{st}")
                    nc.tensor.transpose(tp[:, c * 128:(c + 1) * 128],
                                        xin[:], ident[:])
                    if c == NS // 128 - 1:
                        nc.scalar.copy(out=y0[:], in_=tp[:])
                cur.append(y0)

            # ---- time loop (fully unrolled, stream B emitted 2 eval-phases
            #      behind stream A so the engines always hold anti-phase
            #      work from the other stream) ----
            wmain = (w16, w13, w13, w16)
            wres = (w16r, w13r, w13r, w16r)
            y1t_by_step = {}

            def eval_phase(S_, e):
                """One RK4 eval for stream-state S_: h matmuls, tanh,
                z accumulation, intermediate TTS, early partial combines."""
                st, i = S_["st"], S_["i"]
                if e == 0:
                    Y = S_["Y"]
                    ybh = wpool.tile([D, NS], F32, tag=f"ybh{st}")
                    nc.vector.tensor_scalar(ybh[:], Y[:], b2h[:, i:i + 1],
                                            None, op0=OP.add)
                    ybf = wpool.tile([D, NS], F32, tag=f"ybf{st}")
                    nc.vector.tensor_scalar(ybf[:], Y[:], b2f[:, i:i + 1],
                                            None, op0=OP.add)
                    S_["ybh"], S_["ybf"] = ybh, ybf
                    S_["z1"] = pz1_pool.tile([128, NS], F32, tag=f"z1_{st}", name=f"z1_{st}_{i}")
                    S_["z2"] = pz23_pool.tile([128, NS], F32, tag=f"z23_{st}", name=f"z2_{st}_{i}")
                elif e == 2:
                    S_["z3"] = pz23_pool.tile([128, NS], F32, tag=f"z23_{st}", name=f"z3_{st}_{i}")
                bank = (S_["z1"], S_["z2"], S_.get("z3"), S_["z1"])[e]
                rhs = S_["Y"] if e == 0 else S_["yt"]
                hps = ph_pools[st].tile([128, 4 * NS], F32, tag=f"h{st}")
                a = apool.tile([128, 2 * NS], F32, tag=f"a{st}")
                for c in range(2):
                    reg = hps[:, 2 * c * NS:(2 * c + 1) * NS]
                    nc.tensor.matmul(
                        reg, w1_sb[:, c * 128:(c + 1) * 128],
                        rhs[:], start=True, stop=True)
                    nc.scalar.activation(
                        (_r(a[:, c * NS:(c + 1) * NS]) if use_f32r
                         else a[:, c * NS:(c + 1) * NS]),
                        reg, AF.Tanh, bias=b1col[:, c:c + 1])
                for c in range(2):
                    a_ap = a[:, c * NS:(c + 1) * NS]
                    first = (e != 3) and c == 0
                    last = (e == 3) and c == 1
                    if use_f32r:
                        nc.tensor.matmul(
                            bank[:], _r(wmain[e][:, c, :]), _r(a_ap),
                            start=first, stop=last and not use_res,
                            skip_group_check=True)
                        if use_res:
                            nc.tensor.matmul(
                                bank[:], _r(wres[e][:, c, :]),
                                _r(a_ap), start=False, stop=last,
                                skip_group_check=True)
                    else:
                        nc.tensor.matmul(
                            bank[:], wmain[e][:, c, :], a_ap,
                            start=first, stop=last, skip_group_check=True)
                q, s3, s15 = S_["q"], S_["s3"], S_["s15"]
                if e < 3:
                    yt = wpool.tile([D, NS], F32, tag=f"yt{st}")
                    sc, yb = ((s3, "ybh"), (s15, "ybh"), (s3, "ybf"))[e]
                    nc.vector.scalar_tensor_tensor(
                        yt[:], bank[:], sc, S_[yb][:],
                        op0=OP.mult, op1=OP.add)
                    S_["yt"] = yt
                if e == 1:
                    c1 = wpool.tile([D, NS], F32, tag=f"c{st}")
                    nc.vector.scalar_tensor_tensor(
                        c1[:], S_["z2"][:], q, S_["ybf"][:],
                        op0=OP.mult, op1=OP.add)
                    S_["c1"] = c1
                elif e == 2:
                    c2 = wpool.tile([D, NS], F32, tag=f"c{st}")
                    nc.vector.scalar_tensor_tensor(
                        c2[:], S_["z3"][:], q, S_["c1"][:],
                        op0=OP.mult, op1=OP.add)
                    S_["c2"] = c2
                elif e == 3:
                    ynew = spool.tile([D, NS], F32, tag=f"Y{st}")
                    nc.vector.scalar_tensor_tensor(
                        ynew[:], S_["z1"][:], q, S_["c2"][:],
                        op0=OP.mult, op1=OP.add)
                    S_["ynew"] = ynew
                    tp = pz1_pool.tile([128, NS], F32, tag=f"z1_{st}")
                    for c in range(NS // 128):
                        nc.tensor.transpose(
                            tp[:, c * 128:(c + 1) * 128],
                            ynew[:, c * 128:(c + 1) * 128], ident[:])
                    if i not in y1t_by_step:
                        y1t_by_step[i] = opool.tile([128, N], F32, tag="y1t", name=f"y1t_{i}")
                    y1t = y1t_by_step[i]
                    if st == 0:
                        nc.scalar.copy(out=y1t[:, 0:NS], in_=tp[:])
                    else:
                        nc.vector.tensor_copy(out=y1t[:, NS:2 * NS],
                                              in_=tp[:])

            def new_state(st, i, Y):
                hf = np.float32(h_steps[i])
                return {
                    "st": st, "i": i, "Y": Y,
                    "s3": float(np.float32(3.0) * hf / np.float32(0.05)),
                    "s15": float(np.float32(1.5) * hf / np.float32(0.05)),
                    "q": float(hf / np.float32(0.05)),
                }

            def emit_dma(i):
                y1t = y1t_by_step.pop(i)
                nc.sync.dma_start(
                    out=yout[:, :, i + 1, :].rearrange("s b d -> b s d"),
                    in_=y1t.rearrange("p (s d) -> p s d", s=S))

            nsteps_ = len(h_steps)
            SA = new_state(0, 0, cur[0])
            SB = new_state(1, 0, cur[1])
            SB_prev = None
            for i in range(nsteps_):
                if i > 0:
                    SA = new_state(0, i, SA["ynew"])
                eval_phase(SA, 0)
                eval_phase(SA, 1)
                if i > 0:
                    eval_phase(SB_prev, 3)   # finish B's previous step
                    emit_dma(i - 1)
                    SB = new_state(1, i, SB_prev["ynew"])
                eval_phase(SB, 0)
                eval_phase(SA, 2)
                eval_phase(SB, 1)
                eval_phase(SA, 3)
                eval_phase(SB, 2)
                SB_prev = SB
            eval_phase(SB_prev, 3)
            emit_dma(nsteps_ - 1)

    _split_multiwait_instructions(nc)
    return nc


def _split_multiwait_instructions(nc, max_waits=1):
    """This walrus build rejects >1 sync-wait on CTRL-class instructions
    (Tile's exit Drain carries one wait per live semaphore). N waits on one
    instruction == N single-wait NOPs then the instruction, for same-engine
    in-order execution. Mutate nc.m in place before compile."""
    counter = [0]
    for fn in nc.m.functions:
        for bb in fn.blocks:
            new_instructions = []
            for ins in bb.instructions:
                si = getattr(ins, "sync_info", None)
                if si is not None and si.on_wait and len(si.on_wait) > max_waits:
                    for w in si.on_wait[max_waits:]:
                        counter[0] += 1
                        new_instructions.append(mybir.InstNoOp(
                            name=f"I-drainfix-{counter[0]}",
                            engine=ins.engine, ins=[], outs=[],
                            sync_info=mybir.SyncInfo(on_wait=[w], on_update=[]),
                        ))
                    si.on_wait = si.on_wait[:max_waits]
                new_instructions.append(ins)
            bb.instructions = new_instructions


def kernel(first_point, time_steps_to_predict, W1, b1, W2, b2):
    first_point = np.ascontiguousarray(first_point, dtype=np.float32)
    ts = np.asarray(time_steps_to_predict, dtype=np.float32)
    W1 = np.asarray(W1, dtype=np.float32)
    b1 = np.asarray(b1, dtype=np.float32)
    W2 = np.asarray(W2, dtype=np.float32)
    b2 = np.asarray(b2, dtype=np.float32)

    h_steps = (ts[1:] - ts[:-1]).astype(np.float32)
    key = (h_steps.tobytes(), W2_MODE)
    if key not in _prog_cache:
        _prog_cache[key] = _build(list(h_steps), W2_MODE)
    nc = _prog_cache[key]

    c16 = np.float32(0.05) / np.float32(6.0)
    c13 = np.float32(0.05) / np.float32(3.0)
    w2s16 = np.stack([c16 * W2[0:128, :], c16 * W2[128:256, :]]
                     ).astype(np.float32)
    w2s13 = np.stack([c13 * W2[0:128, :], c13 * W2[128:256, :]]
                     ).astype(np.float32)
    halves = (h_steps * np.float32(0.5)).astype(np.float32)
    b2half = (b2[:, None] * halves[None, :]).astype(np.float32)
    b2full = (b2[:, None] * h_steps[None, :]).astype(np.float32)
    b1col = np.stack([b1[0:128], b1[128:256]], axis=1).astype(np.float32)
    ident = np.eye(128, dtype=np.float32)

    shared = {
        "w1": W1, "b1col": b1col,
        "b2half": b2half, "b2full": b2full, "ident": ident,
    }
    if W2_MODE == "f32r_res":
        m16, m13 = r12(w2s16), r12(w2s13)
        shared["w2s16"], shared["w2s13"] = m16, m13
        shared["w2s16r"] = r12(w2s16 - m16)
        shared["w2s13r"] = r12(w2s13 - m13)
    else:
        shared["w2s16"], shared["w2s13"] = w2s16, w2s13

    in_maps = []
    for i in range(N_CORES):
        m = dict(shared)
        m["x0"] = np.ascontiguousarray(
            first_point[:, i * B_SHARD:(i + 1) * B_SHARD, :])
        in_maps.append(m)

    import os
    trace = os.environ.get("BASS_KERNEL_PROFILE", "") == "1"
    res = run_bass_kernel_spmd(nc, in_maps, list(range(N_CORES)), trace=trace)
    global last_exec_time_ns, last_result
    last_exec_time_ns = res.exec_time_ns
    last_result = res

    out = np.empty((S, B, len(ts), D), dtype=np.float32)
    for i in range(N_CORES):
        out[:, i * B_SHARD:(i + 1) * B_SHARD] = res.results[i]["yout"]
    return out
